# revision 2
# baseline (speedup 1.0000x reference)
"""AdapCNN block on 8 TRN2 NeuronCores.

Strategy (data-parallel over batch, 2 samples per core):
  - The tiny FMN weight-generator MLP (0.8% of FLOPs) runs on host in f32;
    the generated per-sample conv weights are sharded along B to the cores
    (the "shard the generated per-sample weights along B" hint).
  - Each core runs the per-sample 64->64 3x3 VALID conv on its 2 samples.

Conv-as-matmul scheme (75% PE utilization):
  SBUF holds a row-pair layout of x: partitions 0:64 = channels at row r,
  partitions 64:128 = channels at row r+1 (prepared host-side so one
  full-128-partition DMA loads it at full bandwidth).  One matmul per kw with
  lhsT[(t*64+c), (dq*64+o)] = W[o,c,t+dq,kw] * (0.5 if t+dq==1 else 1)
  computes, for PSUM slot j: partitions 0:64 += (kh0 + kh1/2) of output row j,
  partitions 64:128 += (kh1/2 + kh2) of output row j-1.  Output row q =
  psum[0:64, q] + psum[64:128, q+1] + bias: ACT does the cross-partition copy
  of the upper half to SBUF, DVE adds it to the lower half + bias.

  Superblocks of 2 PSUM banks (8 slots -> 7 output rows), 18 per sample,
  psum pool bufs=4 so matmuls never wait on the eviction chain.  Consecutive
  superblocks write the two partition halves of a shared staging tile so the
  output DMA moves 128 partitions at full bandwidth.

Compute dtype bf16 (PSUM accumulates f32), output written f32.
"""
import sys

if '/opt/trn_rl_repo' not in sys.path:
    sys.path.insert(0, '/opt/trn_rl_repo')

import numpy as np
import ml_dtypes

B, CIN, COUT, K = 16, 64, 64, 3
H = W = 128
OH = OW = 126
FC, FMN0, FMN1, G = 512, 512, 512, 4
CNN_PARA = CIN * COUT * K * K + COUT
NCORES = 8
NS = B // NCORES          # samples per core
XROWS = 127               # row-pair layout rows per sample
SB = 7                    # output rows per superblock (8 slots, 2 banks)
NSB = OH // SB            # 18 superblocks per sample

_cached = {}


def _build_module():
    import concourse.mybir as mybir
    import concourse.tile as tile
    from concourse import bacc

    f32 = mybir.dt.float32
    bf16 = mybir.dt.bfloat16

    nc = bacc.Bacc("TRN2", target_bir_lowering=False, debug=False,
                   num_devices=NCORES)
    x_ext = nc.declare_dram_parameter("xh", [NS, 128, XROWS, W], bf16,
                                      isOutput=False)
    wt_ext = nc.declare_dram_parameter("wt", [NS, 3, 128, 128], bf16,
                                       isOutput=False)
    b_ext = nc.declare_dram_parameter("bias", [COUT, NS], f32, isOutput=False)
    y_ext = nc.declare_dram_parameter("y", [NS, COUT, OH, OW], f32,
                                      isOutput=True)

    with tile.TileContext(nc) as tc:
        with tc.tile_pool(name="wts", bufs=1) as wpool, \
             tc.tile_pool(name="xin", bufs=2) as xpool, \
             tc.tile_pool(name="evac", bufs=4) as epool, \
             tc.tile_pool(name="outs", bufs=3) as opool, \
             tc.tile_pool(name="ps", bufs=4, space="PSUM") as pspool:

            wt_sb = wpool.tile([128, NS, 3, 128], bf16)
            for s in range(NS):
                for kw in range(3):
                    nc.sync.dma_start(wt_sb[:, s, kw, :], wt_ext[s, kw, :, :])
            bias_sb = wpool.tile([COUT, NS], f32)
            nc.sync.dma_start(bias_sb[:], b_ext[:])

            for s in range(NS):
                xp = xpool.tile([128, XROWS, W], bf16, tag="xp")
                # chunked load: lets early superblocks start sooner if Tile
                # tracks sub-tile ranges; harmless otherwise
                for r0 in range(0, XROWS, 32):
                    r1 = min(r0 + 32, XROWS)
                    nc.sync.dma_start(xp[:, r0:r1, :], x_ext[s, :, r0:r1, :])

                ob = None
                for bix in range(NSB):
                    j0 = SB * bix
                    ps = pspool.tile([128, SB + 1, 128], f32, tag="ps")
                    for kw in range(3):
                        for kb in range(2):
                            ss = 4 * kb
                            nc.tensor.matmul(
                                ps[:, ss:ss + 4, 0:OW],
                                wt_sb[:, s, kw, :],
                                xp[:, j0 + ss:j0 + ss + 4, kw:kw + OW],
                                start=(kw == 0), stop=(kw == 2))
                    tmp = epool.tile([64, SB, OW], f32, tag="tmp")
                    nc.scalar.copy(tmp[:], ps[64:128, 1:1 + SB, 0:OW])
                    dq = bix % 2
                    if dq == 0:
                        ob = opool.tile([128, SB, OW], f32, tag="ob")
                    nc.vector.scalar_tensor_tensor(
                        ob[64 * dq:64 * dq + 64, :, :],
                        ps[0:64, 0:SB, 0:OW],
                        bias_sb[:, s:s + 1],
                        tmp[:],
                        mybir.AluOpType.add,
                        mybir.AluOpType.add)
                    if dq == 1:
                        pair = bix // 2
                        yv = y_ext[s, :, 14 * pair:14 * pair + 14, :] \
                            .rearrange("c (q r) w -> q c r w", q=2)
                        nc.sync.dma_start(yv, ob[:])
    nc.compile()
    return nc


def _fmn_host(fc_in, w1, b1, w2, b2, w3, b3):
    h = np.maximum(fc_in @ w1.T + b1, 0.0)
    h = np.maximum(h @ w2.T + b2, 0.0)
    hg = h.reshape(h.shape[0], G, FMN1 // G)
    o = np.einsum('bgi,goi->bgo', hg, w3,
                  dtype=np.float32).reshape(h.shape[0], -1) + b3
    return np.maximum(o, 0.0)


def _prep_inputs(x, fc_in, w1, b1, w2, b2, w3, b3):
    wb = _fmn_host(fc_in, w1, b1, w2, b2, w3, b3)          # [B, CNN_PARA]
    weight = wb[:, :-COUT].reshape(B, COUT, CIN, K, K)
    bias = wb[:, -COUT:]                                   # [B, COUT]

    # lhsT[s, kw, t*64+c, dq*64+o] = weight[s, o, c, t+dq, kw] * scale
    wk = weight.transpose(0, 4, 3, 2, 1)                   # [B, kw, kh, c, o]
    lhsT = np.empty((B, 3, 128, 128), np.float32)
    for t in (0, 1):
        for dq in (0, 1):
            kh = t + dq
            sc = 0.5 if kh == 1 else 1.0
            lhsT[:, :, t * 64:t * 64 + 64, dq * 64:dq * 64 + 64] = \
                wk[:, :, kh] * sc
    lhsT = lhsT.astype(ml_dtypes.bfloat16)

    xb = x.astype(ml_dtypes.bfloat16)                      # [B, 64, 128, 128]
    xpair = np.empty((B, 128, XROWS, W), ml_dtypes.bfloat16)
    xpair[:, :64] = xb[:, :, 0:XROWS]
    xpair[:, 64:] = xb[:, :, 1:XROWS + 1]

    in_maps = []
    for c in range(NCORES):
        s0 = NS * c
        in_maps.append({
            "xh": np.ascontiguousarray(xpair[s0:s0 + NS]),
            "wt": np.ascontiguousarray(lhsT[s0:s0 + NS]),
            "bias": np.ascontiguousarray(bias[s0:s0 + NS].T),
        })
    return in_maps


def kernel(x, fc_in, w1, b1, w2, b2, w3, b3, splits):
    from concourse.bass_utils import run_bass_kernel_spmd

    x = np.asarray(x, np.float32)
    args = [np.asarray(a, np.float32)
            for a in (fc_in, w1, b1, w2, b2, w3, b3)]
    in_maps = _prep_inputs(x, *args)

    if 'nc' not in _cached:
        _cached['nc'] = _build_module()
    nc = _cached['nc']

    res = run_bass_kernel_spmd(nc, in_maps, core_ids=list(range(NCORES)))

    out = np.empty((B * COUT, OH, OW), np.float32)
    for c in range(NCORES):
        y = res.results[c]["y"]                            # [NS, COUT, OH, OW]
        out[NS * COUT * c:NS * COUT * (c + 1)] = \
            np.asarray(y, np.float32).reshape(NS * COUT, OH, OW)
    return out.reshape(1, B * COUT, 1, OH, OW)


# revision 3
# speedup vs baseline: 2.4535x; 2.4535x over previous
"""AdapCNN block on 8 TRN2 NeuronCores.

Strategy (data-parallel over batch, 2 samples per core):
  - The tiny FMN weight-generator MLP (0.8% of FLOPs) runs on host in f32;
    the generated per-sample conv weights are sharded along B to the cores
    (the "shard the generated per-sample weights along B" hint).
  - Each core runs the per-sample 64->64 3x3 VALID conv on its 2 samples.

Conv-as-matmul scheme (75% PE utilization):
  SBUF holds a row-pair layout of x: partitions 0:64 = channels at row r,
  partitions 64:128 = channels at row r+1 (prepared host-side so one
  full-128-partition DMA loads it at full bandwidth).  One matmul per kw with
  lhsT[(t*64+c), (dq*64+o)] = W[o,c,t+dq,kw] * (0.5 if t+dq==1 else 1)
  computes, for PSUM slot j: partitions 0:64 += (kh0 + kh1/2) of output row j,
  partitions 64:128 += (kh1/2 + kh2) of output row j-1.  Output row q =
  psum[0:64, q] + psum[64:128, q+1] + bias: ACT does the cross-partition copy
  of the upper half to SBUF, DVE adds it to the lower half + bias.

  Superblocks of 2 PSUM banks (8 slots -> 7 output rows), 18 per sample,
  psum pool bufs=4 so matmuls never wait on the eviction chain.  Consecutive
  superblocks write the two partition halves of a shared staging tile so the
  output DMA moves 128 partitions at full bandwidth.

Compute dtype bf16 (PSUM accumulates f32), output written f32.
"""
import sys

if '/opt/trn_rl_repo' not in sys.path:
    sys.path.insert(0, '/opt/trn_rl_repo')

import numpy as np
import ml_dtypes

B, CIN, COUT, K = 16, 64, 64, 3
H = W = 128
OH = OW = 126
FC, FMN0, FMN1, G = 512, 512, 512, 4
CNN_PARA = CIN * COUT * K * K + COUT
NCORES = 8
NS = B // NCORES          # samples per core
XROWS = 127               # row-pair layout rows per sample
SB = 7                    # output rows per superblock (8 slots, 2 banks)
NSB = OH // SB            # 18 superblocks per sample

_cached = {}


def _build_module():
    import concourse.mybir as mybir
    import concourse.tile as tile
    from concourse import bacc

    f32 = mybir.dt.float32
    bf16 = mybir.dt.bfloat16

    nc = bacc.Bacc("TRN2", target_bir_lowering=False, debug=False,
                   num_devices=NCORES)
    x_ext = nc.declare_dram_parameter("xh", [NS, 128, XROWS, W], bf16,
                                      isOutput=False)
    wt_ext = nc.declare_dram_parameter("wt", [NS, 3, 128, 128], bf16,
                                       isOutput=False)
    b_ext = nc.declare_dram_parameter("bias", [COUT, NS], f32, isOutput=False)
    y_ext = nc.declare_dram_parameter("y", [NS, COUT, OH, OW], f32,
                                      isOutput=True)

    with tile.TileContext(nc) as tc:
        with tc.tile_pool(name="wts", bufs=1) as wpool, \
             tc.tile_pool(name="xin", bufs=2) as xpool, \
             tc.tile_pool(name="evac", bufs=4) as epool, \
             tc.tile_pool(name="outs", bufs=3) as opool, \
             tc.tile_pool(name="ps", bufs=4, space="PSUM") as pspool:

            wt_sb = wpool.tile([128, NS, 3, 128], bf16)
            for s in range(NS):
                for kw in range(3):
                    nc.sync.dma_start(wt_sb[:, s, kw, :], wt_ext[s, kw, :, :])
            bias_sb = wpool.tile([COUT, NS], f32)
            nc.sync.dma_start(bias_sb[:], b_ext[:])

            for s in range(NS):
                xp = xpool.tile([128, XROWS, W], bf16, tag="xp")
                # chunked load: lets early superblocks start sooner if Tile
                # tracks sub-tile ranges; harmless otherwise
                for r0 in range(0, XROWS, 32):
                    r1 = min(r0 + 32, XROWS)
                    nc.sync.dma_start(xp[:, r0:r1, :], x_ext[s, :, r0:r1, :])

                ob = None
                for bix in range(NSB):
                    j0 = SB * bix
                    ps = pspool.tile([128, SB + 1, 128], f32, tag="ps")
                    for kw in range(3):
                        for kb in range(2):
                            ss = 4 * kb
                            nc.tensor.matmul(
                                ps[:, ss:ss + 4, 0:OW],
                                wt_sb[:, s, kw, :],
                                xp[:, j0 + ss:j0 + ss + 4, kw:kw + OW],
                                start=(kw == 0), stop=(kw == 2))
                    tmp = epool.tile([64, SB, OW], f32, tag="tmp")
                    nc.scalar.copy(tmp[:], ps[64:128, 1:1 + SB, 0:OW])
                    dq = bix % 2
                    if dq == 0:
                        ob = opool.tile([128, SB, OW], f32, tag="ob")
                    nc.vector.scalar_tensor_tensor(
                        ob[64 * dq:64 * dq + 64, :, :],
                        ps[0:64, 0:SB, 0:OW],
                        bias_sb[:, s:s + 1],
                        tmp[:],
                        mybir.AluOpType.add,
                        mybir.AluOpType.add)
                    if dq == 1:
                        pair = bix // 2
                        r0 = 14 * pair
                        nc.sync.dma_start(
                            y_ext[s, :, r0:r0 + SB, :], ob[0:64, :, :])
                        nc.sync.dma_start(
                            y_ext[s, :, r0 + SB:r0 + 2 * SB, :],
                            ob[64:128, :, :])
    nc.compile()
    return nc


def _fmn_host(fc_in, w1, b1, w2, b2, w3, b3):
    h = np.maximum(fc_in @ w1.T + b1, 0.0)
    h = np.maximum(h @ w2.T + b2, 0.0)
    hg = h.reshape(h.shape[0], G, FMN1 // G)
    o = np.einsum('bgi,goi->bgo', hg, w3,
                  dtype=np.float32).reshape(h.shape[0], -1) + b3
    return np.maximum(o, 0.0)


def _prep_inputs(x, fc_in, w1, b1, w2, b2, w3, b3):
    wb = _fmn_host(fc_in, w1, b1, w2, b2, w3, b3)          # [B, CNN_PARA]
    weight = wb[:, :-COUT].reshape(B, COUT, CIN, K, K)
    bias = wb[:, -COUT:]                                   # [B, COUT]

    # lhsT[s, kw, t*64+c, dq*64+o] = weight[s, o, c, t+dq, kw] * scale
    wk = weight.transpose(0, 4, 3, 2, 1)                   # [B, kw, kh, c, o]
    lhsT = np.empty((B, 3, 128, 128), np.float32)
    for t in (0, 1):
        for dq in (0, 1):
            kh = t + dq
            sc = 0.5 if kh == 1 else 1.0
            lhsT[:, :, t * 64:t * 64 + 64, dq * 64:dq * 64 + 64] = \
                wk[:, :, kh] * sc
    lhsT = lhsT.astype(ml_dtypes.bfloat16)

    xb = x.astype(ml_dtypes.bfloat16)                      # [B, 64, 128, 128]
    xpair = np.empty((B, 128, XROWS, W), ml_dtypes.bfloat16)
    xpair[:, :64] = xb[:, :, 0:XROWS]
    xpair[:, 64:] = xb[:, :, 1:XROWS + 1]

    in_maps = []
    for c in range(NCORES):
        s0 = NS * c
        in_maps.append({
            "xh": np.ascontiguousarray(xpair[s0:s0 + NS]),
            "wt": np.ascontiguousarray(lhsT[s0:s0 + NS]),
            "bias": np.ascontiguousarray(bias[s0:s0 + NS].T),
        })
    return in_maps


def kernel(x, fc_in, w1, b1, w2, b2, w3, b3, splits):
    from concourse.bass_utils import run_bass_kernel_spmd

    x = np.asarray(x, np.float32)
    args = [np.asarray(a, np.float32)
            for a in (fc_in, w1, b1, w2, b2, w3, b3)]
    in_maps = _prep_inputs(x, *args)

    if 'nc' not in _cached:
        _cached['nc'] = _build_module()
    nc = _cached['nc']

    res = run_bass_kernel_spmd(nc, in_maps, core_ids=list(range(NCORES)))

    out = np.empty((B * COUT, OH, OW), np.float32)
    for c in range(NCORES):
        y = res.results[c]["y"]                            # [NS, COUT, OH, OW]
        out[NS * COUT * c:NS * COUT * (c + 1)] = \
            np.asarray(y, np.float32).reshape(NS * COUT, OH, OW)
    return out.reshape(1, B * COUT, 1, OH, OW)


# revision 7
# speedup vs baseline: 2.7301x; 1.1127x over previous
"""AdapCNN block on 8 TRN2 NeuronCores.

Strategy (data-parallel over batch, 2 samples per core):
  - The tiny FMN weight-generator MLP (0.8% of FLOPs) runs on host in f32;
    the generated per-sample conv weights are sharded along B to the cores
    (the "shard the generated per-sample weights along B" hint).
  - Each core runs the per-sample 64->64 3x3 VALID conv on its 2 samples.

Conv-as-matmul scheme (75% PE utilization):
  SBUF holds a row-pair layout of x: partitions 0:64 = channels at row r,
  partitions 64:128 = channels at row r+1 (prepared host-side so one
  full-128-partition DMA loads it at full bandwidth).  One matmul per kw with
  lhsT[(t*64+c), (dq*64+o)] = W[o,c,t+dq,kw] * (0.5 if t+dq==1 else 1)
  computes, for PSUM slot j: partitions 0:64 += (kh0 + kh1/2) of output row j,
  partitions 64:128 += (kh1/2 + kh2) of output row j-1.  Output row q =
  psum[0:64, q] + psum[64:128, q+1] + bias: ACT does the cross-partition copy
  of the upper half to SBUF, DVE adds it to the lower half + bias.

  Superblocks of 2 PSUM banks (8 slots -> 7 output rows), 18 per sample,
  psum pool bufs=4 so matmuls never wait on the eviction chain.  Consecutive
  superblocks write the two partition halves of a shared staging tile so the
  output DMA moves 128 partitions at full bandwidth.

Compute dtype bf16 (PSUM accumulates f32), output written f32.
"""
import sys

if '/opt/trn_rl_repo' not in sys.path:
    sys.path.insert(0, '/opt/trn_rl_repo')

import numpy as np
import ml_dtypes

B, CIN, COUT, K = 16, 64, 64, 3
H = W = 128
OH = OW = 126
FC, FMN0, FMN1, G = 512, 512, 512, 4
CNN_PARA = CIN * COUT * K * K + COUT
NCORES = 8
NS = B // NCORES          # samples per core
XROWS = 127               # row-pair layout rows per sample
SB = 7                    # output rows per superblock (8 slots, 2 banks)
NSB = OH // SB            # 18 superblocks per sample

_cached = {}


def _build_module():
    import concourse.mybir as mybir
    import concourse.tile as tile
    from concourse import bacc

    f32 = mybir.dt.float32
    bf16 = mybir.dt.bfloat16

    nc = bacc.Bacc("TRN2", target_bir_lowering=False, debug=False,
                   num_devices=NCORES)
    x_ext = nc.declare_dram_parameter("xh", [NS, 128, XROWS, W], bf16,
                                      isOutput=False)
    wt_ext = nc.declare_dram_parameter("wt", [NS, 3, 128, 128], bf16,
                                       isOutput=False)
    b_ext = nc.declare_dram_parameter("bias", [COUT, NS], f32, isOutput=False)
    y_ext = nc.declare_dram_parameter("y", [NS, COUT, OH, OW], bf16,
                                      isOutput=True)

    with tile.TileContext(nc) as tc:
        with tc.tile_pool(name="wts", bufs=1) as wpool, \
             tc.tile_pool(name="xin", bufs=5) as xpool, \
             tc.tile_pool(name="evac", bufs=4) as epool, \
             tc.tile_pool(name="outs", bufs=3) as opool, \
             tc.tile_pool(name="ps", bufs=4, space="PSUM") as pspool:

            wt_sb = wpool.tile([128, NS, 3, 128], bf16)
            for s in range(NS):
                for kw in range(3):
                    nc.sync.dma_start(wt_sb[:, s, kw, :], wt_ext[s, kw, :, :])
            bias_sb = wpool.tile([COUT, NS], f32)
            nc.sync.dma_start(bias_sb[:], b_ext[:])

            # x row bands: band k of sample s covers superblocks 6k..6k+5,
            # which need rows [42k, 42k+43) of the row-pair layout
            BAND = 43

            for s in range(NS):
                xbands = []
                for k in range(3):
                    xb = xpool.tile([128, BAND, W], bf16, tag="xp")
                    nc.sync.dma_start(xb[:],
                                      x_ext[s, :, 42 * k:42 * k + BAND, :])
                    xbands.append(xb)

                ob = None
                for bix in range(NSB):
                    j0 = SB * bix
                    k = bix // 6
                    lj0 = j0 - 42 * k
                    xb = xbands[k]
                    ps = pspool.tile([128, SB + 1, 128], f32, tag="ps")
                    for kw in range(3):
                        for kb in range(2):
                            ss = 4 * kb
                            nc.tensor.matmul(
                                ps[:, ss:ss + 4, 0:OW],
                                wt_sb[:, s, kw, :],
                                xb[:, lj0 + ss:lj0 + ss + 4, kw:kw + OW],
                                start=(kw == 0), stop=(kw == 2))
                    tmp = epool.tile([64, SB, OW], f32, tag="tmp")
                    nc.scalar.copy(tmp[:], ps[64:128, 1:1 + SB, 0:OW])
                    dq = bix % 2
                    if dq == 0:
                        ob = opool.tile([128, SB, OW], bf16, tag="ob")
                    nc.vector.scalar_tensor_tensor(
                        ob[64 * dq:64 * dq + 64, :, :],
                        ps[0:64, 0:SB, 0:OW],
                        bias_sb[:, s:s + 1],
                        tmp[:],
                        mybir.AluOpType.add,
                        mybir.AluOpType.add)
                    if dq == 1:
                        pair = bix // 2
                        r0 = 14 * pair
                        nc.sync.dma_start(
                            y_ext[s, :, r0:r0 + SB, :], ob[0:64, :, :])
                        nc.sync.dma_start(
                            y_ext[s, :, r0 + SB:r0 + 2 * SB, :],
                            ob[64:128, :, :])
    nc.compile()
    return nc


def _fmn_host(fc_in, w1, b1, w2, b2, w3, b3):
    h = np.maximum(fc_in @ w1.T + b1, 0.0)
    h = np.maximum(h @ w2.T + b2, 0.0)
    hg = h.reshape(h.shape[0], G, FMN1 // G)
    o = np.einsum('bgi,goi->bgo', hg, w3,
                  dtype=np.float32).reshape(h.shape[0], -1) + b3
    return np.maximum(o, 0.0)


def _prep_inputs(x, fc_in, w1, b1, w2, b2, w3, b3):
    wb = _fmn_host(fc_in, w1, b1, w2, b2, w3, b3)          # [B, CNN_PARA]
    weight = wb[:, :-COUT].reshape(B, COUT, CIN, K, K)
    bias = wb[:, -COUT:]                                   # [B, COUT]

    # lhsT[s, kw, t*64+c, dq*64+o] = weight[s, o, c, t+dq, kw] * scale
    wk = weight.transpose(0, 4, 3, 2, 1)                   # [B, kw, kh, c, o]
    lhsT = np.empty((B, 3, 128, 128), np.float32)
    for t in (0, 1):
        for dq in (0, 1):
            kh = t + dq
            sc = 0.5 if kh == 1 else 1.0
            lhsT[:, :, t * 64:t * 64 + 64, dq * 64:dq * 64 + 64] = \
                wk[:, :, kh] * sc
    lhsT = lhsT.astype(ml_dtypes.bfloat16)

    xb = x.astype(ml_dtypes.bfloat16)                      # [B, 64, 128, 128]
    xpair = np.empty((B, 128, XROWS, W), ml_dtypes.bfloat16)
    xpair[:, :64] = xb[:, :, 0:XROWS]
    xpair[:, 64:] = xb[:, :, 1:XROWS + 1]

    in_maps = []
    for c in range(NCORES):
        s0 = NS * c
        in_maps.append({
            "xh": np.ascontiguousarray(xpair[s0:s0 + NS]),
            "wt": np.ascontiguousarray(lhsT[s0:s0 + NS]),
            "bias": np.ascontiguousarray(bias[s0:s0 + NS].T),
        })
    return in_maps


def kernel(x, fc_in, w1, b1, w2, b2, w3, b3, splits):
    from concourse.bass_utils import run_bass_kernel_spmd

    x = np.asarray(x, np.float32)
    args = [np.asarray(a, np.float32)
            for a in (fc_in, w1, b1, w2, b2, w3, b3)]
    in_maps = _prep_inputs(x, *args)

    if 'nc' not in _cached:
        _cached['nc'] = _build_module()
    nc = _cached['nc']

    res = run_bass_kernel_spmd(nc, in_maps, core_ids=list(range(NCORES)))

    out = np.empty((B * COUT, OH, OW), np.float32)
    for c in range(NCORES):
        y = res.results[c]["y"]                            # [NS, COUT, OH, OW]
        out[NS * COUT * c:NS * COUT * (c + 1)] = \
            np.asarray(y, np.float32).reshape(NS * COUT, OH, OW)
    return out.reshape(1, B * COUT, 1, OH, OW)


# revision 12
# speedup vs baseline: 2.8042x; 1.0271x over previous
"""AdapCNN block on 8 TRN2 NeuronCores.

Strategy (data-parallel over batch, 2 samples per core):
  - The tiny FMN weight-generator MLP (0.8% of FLOPs) runs on host in f32;
    the generated per-sample conv weights are sharded along B to the cores
    (the "shard the generated per-sample weights along B" hint).
  - Each core runs the per-sample 64->64 3x3 VALID conv on its 2 samples.

Conv-as-matmul scheme (75% PE utilization):
  SBUF holds a row-pair layout of x: partitions 0:64 = channels at row r,
  partitions 64:128 = channels at row r+1 (prepared host-side so one
  full-128-partition DMA loads it at full bandwidth).  One matmul per kw with
  lhsT[(t*64+c), (dq*64+o)] = W[o,c,t+dq,kw] * (0.5 if t+dq==1 else 1)
  computes, for PSUM slot j: partitions 0:64 += (kh0 + kh1/2) of output row j,
  partitions 64:128 += (kh1/2 + kh2) of output row j-1.  Output row q =
  psum[0:64, q] + psum[64:128, q+1] + bias: ACT does the cross-partition copy
  of the upper half to SBUF, DVE adds it to the lower half + bias.

  Superblocks of 2 PSUM banks (8 slots -> 7 output rows), 18 per sample,
  psum pool bufs=4 so matmuls never wait on the eviction chain.  Consecutive
  superblocks write the two partition halves of a shared staging tile so the
  output DMA moves 128 partitions at full bandwidth.

Compute dtype bf16 (PSUM accumulates f32), output written f32.
"""
import sys

if '/opt/trn_rl_repo' not in sys.path:
    sys.path.insert(0, '/opt/trn_rl_repo')

import numpy as np
import ml_dtypes

B, CIN, COUT, K = 16, 64, 64, 3
H = W = 128
OH = OW = 126
FC, FMN0, FMN1, G = 512, 512, 512, 4
CNN_PARA = CIN * COUT * K * K + COUT
NCORES = 8
NS = B // NCORES          # samples per core
XROWS = 127               # row-pair layout rows per sample
SB = 7                    # output rows per superblock (8 slots, 2 banks)
NSB = OH // SB            # 18 superblocks per sample

_cached = {}


def _build_module():
    import concourse.mybir as mybir
    import concourse.tile as tile
    from concourse import bacc

    f32 = mybir.dt.float32
    bf16 = mybir.dt.bfloat16

    nc = bacc.Bacc("TRN2", target_bir_lowering=False, debug=False,
                   num_devices=NCORES)
    x_ext = nc.declare_dram_parameter("xh", [NS, 128, XROWS, W], bf16,
                                      isOutput=False)
    # weights pre-packed host-side as [partition, s, kw, m] -> one DMA
    wt_ext = nc.declare_dram_parameter("wt", [128, NS * 3 * 128], bf16,
                                       isOutput=False)
    b_ext = nc.declare_dram_parameter("bias", [COUT, NS], f32, isOutput=False)
    y_ext = nc.declare_dram_parameter("y", [NS, COUT, OH, OW], bf16,
                                      isOutput=True)

    with tile.TileContext(nc) as tc:
        with tc.tile_pool(name="wts", bufs=1) as wpool, \
             tc.tile_pool(name="xin", bufs=5) as xpool, \
             tc.tile_pool(name="evac", bufs=4) as epool, \
             tc.tile_pool(name="outs", bufs=3) as opool, \
             tc.tile_pool(name="ps", bufs=4, space="PSUM") as pspool:

            wt_sb = wpool.tile([128, NS, 3, 128], bf16)
            nc.sync.dma_start(
                wt_sb[:].rearrange("p s k m -> p (s k m)"), wt_ext[:])
            bias_sb = wpool.tile([COUT, NS], f32)
            nc.gpsimd.dma_start(bias_sb[:], b_ext[:])

            # x row bands: band k of sample s covers superblocks 6k..6k+5,
            # which need rows [42k, 42k+43) of the row-pair layout
            BAND = 43

            for s in range(NS):
                xbands = []
                for k in range(3):
                    xb = xpool.tile([128, BAND, W], bf16, tag="xp")
                    nc.sync.dma_start(xb[:],
                                      x_ext[s, :, 42 * k:42 * k + BAND, :])
                    xbands.append(xb)

                ob = None
                for bix in range(NSB):
                    j0 = SB * bix
                    k = bix // 6
                    lj0 = j0 - 42 * k
                    xb = xbands[k]
                    ps = pspool.tile([128, SB + 1, 128], f32, tag="ps")
                    for kw in range(3):
                        for kb in range(2):
                            ss = 4 * kb
                            nc.tensor.matmul(
                                ps[:, ss:ss + 4, 0:OW],
                                wt_sb[:, s, kw, :],
                                xb[:, lj0 + ss:lj0 + ss + 4, kw:kw + OW],
                                start=(kw == 0), stop=(kw == 2))
                    tmp = epool.tile([64, SB, OW], f32, tag="tmp")
                    nc.scalar.copy(tmp[:], ps[64:128, 1:1 + SB, 0:OW])
                    dq = bix % 2
                    if dq == 0:
                        ob = opool.tile([128, SB, OW], bf16, tag="ob")
                    nc.vector.scalar_tensor_tensor(
                        ob[64 * dq:64 * dq + 64, :, :],
                        ps[0:64, 0:SB, 0:OW],
                        bias_sb[:, s:s + 1],
                        tmp[:],
                        mybir.AluOpType.add,
                        mybir.AluOpType.add)
                    if dq == 1:
                        pair = bix // 2
                        r0 = 14 * pair
                        nc.sync.dma_start(
                            y_ext[s, :, r0:r0 + SB, :], ob[0:64, :, :])
                        nc.gpsimd.dma_start(
                            y_ext[s, :, r0 + SB:r0 + 2 * SB, :],
                            ob[64:128, :, :])
    nc.compile()
    return nc


def _fmn_host(fc_in, w1, b1, w2, b2, w3, b3):
    h = np.maximum(fc_in @ w1.T + b1, 0.0)
    h = np.maximum(h @ w2.T + b2, 0.0)
    hg = h.reshape(h.shape[0], G, FMN1 // G)
    o = np.einsum('bgi,goi->bgo', hg, w3,
                  dtype=np.float32).reshape(h.shape[0], -1) + b3
    return np.maximum(o, 0.0)


def _prep_inputs(x, fc_in, w1, b1, w2, b2, w3, b3):
    wb = _fmn_host(fc_in, w1, b1, w2, b2, w3, b3)          # [B, CNN_PARA]
    weight = wb[:, :-COUT].reshape(B, COUT, CIN, K, K)
    bias = wb[:, -COUT:]                                   # [B, COUT]

    # lhsT[s, kw, t*64+c, dq*64+o] = weight[s, o, c, t+dq, kw] * scale
    wk = weight.transpose(0, 4, 3, 2, 1)                   # [B, kw, kh, c, o]
    lhsT = np.empty((B, 3, 128, 128), np.float32)
    for t in (0, 1):
        for dq in (0, 1):
            kh = t + dq
            sc = 0.5 if kh == 1 else 1.0
            lhsT[:, :, t * 64:t * 64 + 64, dq * 64:dq * 64 + 64] = \
                wk[:, :, kh] * sc
    lhsT = lhsT.astype(ml_dtypes.bfloat16)
    # device layout: [partition, s, kw, m]
    lhsT = lhsT.transpose(2, 0, 1, 3)                      # [128, B, 3, 128]

    xb = x.astype(ml_dtypes.bfloat16)                      # [B, 64, 128, 128]
    xpair = np.empty((B, 128, XROWS, W), ml_dtypes.bfloat16)
    xpair[:, :64] = xb[:, :, 0:XROWS]
    xpair[:, 64:] = xb[:, :, 1:XROWS + 1]

    in_maps = []
    for c in range(NCORES):
        s0 = NS * c
        in_maps.append({
            "xh": np.ascontiguousarray(xpair[s0:s0 + NS]),
            "wt": np.ascontiguousarray(
                lhsT[:, s0:s0 + NS].reshape(128, NS * 3 * 128)),
            "bias": np.ascontiguousarray(bias[s0:s0 + NS].T),
        })
    return in_maps


def kernel(x, fc_in, w1, b1, w2, b2, w3, b3, splits):
    from concourse.bass_utils import run_bass_kernel_spmd

    x = np.asarray(x, np.float32)
    args = [np.asarray(a, np.float32)
            for a in (fc_in, w1, b1, w2, b2, w3, b3)]
    in_maps = _prep_inputs(x, *args)

    if 'nc' not in _cached:
        _cached['nc'] = _build_module()
    nc = _cached['nc']

    res = run_bass_kernel_spmd(nc, in_maps, core_ids=list(range(NCORES)))

    out = np.empty((B * COUT, OH, OW), np.float32)
    for c in range(NCORES):
        y = res.results[c]["y"]                            # [NS, COUT, OH, OW]
        out[NS * COUT * c:NS * COUT * (c + 1)] = \
            np.asarray(y, np.float32).reshape(NS * COUT, OH, OW)
    return out.reshape(1, B * COUT, 1, OH, OW)
